# revision 9
# baseline (speedup 1.0000x reference)
"""MixHop GNN kernel for 8 Trainium2 NeuronCores (Bass/Tile).

Strategy
--------
Rows (nodes) are sharded 8 ways. The six SpMMs of the reference are
algebraically folded to four over the same normalized adjacency A
(A^j (x W_j) == (A^j x) W_j, and the final projection is folded into the
second MixHop: out = h M0 + A(h M1 + A(h M2)) + bias terms, with
M_j = l2_w[j] @ fp_w[j*128:(j+1)*128]).

Each SpMM is computed per output row-tile of 128 rows as a sequence of
128-edge chunks: an indirect DMA (dma_gather) fetches the 512B source rows
G = h[col] into SBUF keyed by int16 indices, the DVE builds a val-carrying
one-hot S[e, r] = val[e] * (row_local[e] == r) with one fused tensor_scalar
(iota is_equal row_local, then mult val), and the PE accumulates
psum[feat, row] += G^T S into PSUM. Activations live in transposed layout
[feat, row] so dense layers use stored weights as lhsT directly; BN stats
become free-axis DVE reductions. Full h matrices needed by later hops are
re-assembled with AllGather collectives (rows-major via PE transpose).

The graph structure (CSR by row-tile, split lo/hi source halves for int16
indexing, padded to 128-edge chunks with row_local=-1 kills) is identical
for all four SpMMs, so one set of index tensors is loaded once and reused.
"""

import os
import numpy as np

N = 50000
E = 1600000
F = 128
NCORES = 8
RPC = N // NCORES            # 6250 rows per core
TI = 128                     # row tile
NT = (RPC + TI - 1) // TI    # 49 tiles per core
RPAD = NT * TI               # 6272 padded rows per core
HALF = 32768                 # int16 gather index limit
BN_EPS = 1e-5
MAXG = 8                     # max chunks (of 128 edges) per dma_gather call
INV_N = 1.0 / N

_COMPILED = {}
LAST_EXEC_NS = None


# ----------------------------------------------------------------------------
# Host-side graph preprocessing (index/sharding work only)
# ----------------------------------------------------------------------------

def _preprocess(edge_index):
    row = np.concatenate([np.asarray(edge_index[0]), np.arange(N, dtype=np.int64)])
    col = np.concatenate([np.asarray(edge_index[1]), np.arange(N, dtype=np.int64)])
    row = row.astype(np.int64)
    col = col.astype(np.int64)
    deg = np.bincount(col, minlength=N).astype(np.float64)
    with np.errstate(divide="ignore"):
        dinv = np.where(deg > 0, deg ** -0.5, 0.0)
    val = (dinv[row] * dinv[col]).astype(np.float32)
    # s_j = A^j @ 1  (for bias terms)
    s1 = np.bincount(row, weights=val.astype(np.float64), minlength=N)
    s2 = np.bincount(row, weights=val.astype(np.float64) * s1[col], minlength=N)
    svec = np.stack([np.ones(N), s1, s2]).astype(np.float32)  # [3, N]

    # Per-core CSR-ish structure, grouped by (row-tile, source half)
    per_core = []  # per core: dict (t,h) -> (idx16, rl, vv)
    counts = np.zeros((NCORES, NT, 2), dtype=np.int64)
    for c in range(NCORES):
        lo, hi = c * RPC, (c + 1) * RPC
        m = (row >= lo) & (row < hi)
        r_loc = (row[m] - lo).astype(np.int64)
        cl = col[m]
        vl = val[m]
        tid = r_loc // TI
        rit = (r_loc % TI).astype(np.float32)
        half = (cl >= HALF).astype(np.int64)
        # order edges by (tile, half) with a stable counting sort
        key = tid * 2 + half
        order = np.argsort(key, kind="stable")
        key_s = key[order]
        cl_s, rit_s, vl_s = cl[order], rit[order], vl[order]
        bounds = np.searchsorted(key_s, np.arange(NT * 2 + 1))
        groups = {}
        for t in range(NT):
            for h in range(2):
                a, b = bounds[t * 2 + h], bounds[t * 2 + h + 1]
                idx16 = (cl_s[a:b] - h * HALF).astype(np.int16)
                groups[(t, h)] = (idx16, rit_s[a:b], vl_s[a:b])
                counts[c, t, h] = b - a
        per_core.append(groups)

    # shared chunk counts (max over cores), padded to 128-edge chunks
    nch = np.maximum((counts.max(axis=0) + TI - 1) // TI, 0).astype(np.int64)  # [NT, 2]

    # program-constant call list: (t, h, chunk_off, nch_call, n_prev_chunks_in_tile)
    calls = []
    chunk_off = 0
    tile_chunks = []  # per tile: total chunks
    for t in range(NT):
        tot = 0
        for h in range(2):
            n = int(nch[t, h])
            k = 0
            while k < n:
                nck = min(MAXG, n - k)
                calls.append((t, h, chunk_off + tot + k, nck))
                k += nck
            tot += n
        tile_chunks.append(tot)
        chunk_off += tot
    C_total = chunk_off
    S_total = C_total * 8  # int16 idx columns (16 idx per column)

    # per-core flat index tensors
    idx_np = np.zeros((NCORES, 128, S_total), dtype=np.int16)
    rl_np = np.full((NCORES, 128, C_total), -1.0, dtype=np.float32)
    val_np = np.zeros((NCORES, 128, C_total), dtype=np.float32)
    for c in range(NCORES):
        co = 0
        for t in range(NT):
            for h in range(2):
                n = int(nch[t, h])
                if n == 0:
                    continue
                idx16, rl, vv = per_core[c][(t, h)]
                L = n * TI
                ib = np.zeros(L, dtype=np.int16)
                rb = np.full(L, -1.0, dtype=np.float32)
                vb = np.zeros(L, dtype=np.float32)
                ib[: len(idx16)] = idx16
                rb[: len(rl)] = rl
                vb[: len(vv)] = vv
                # idx layout: idx i -> [i % 16, off + i // 16], replicated x8
                icols = ib.reshape(-1, 16).T  # [16, L/16]
                idx_np[c, :, co * 8 : co * 8 + L // 16] = np.tile(icols, (8, 1))
                rl_np[c, :, co : co + n] = rb.reshape(n, TI).T
                val_np[c, :, co : co + n] = vb.reshape(n, TI).T
                co += n
    return {
        "calls": calls,
        "tile_chunks": tile_chunks,
        "nch": nch,
        "C_total": C_total,
        "S_total": S_total,
        "idx": idx_np,
        "rl": rl_np,
        "val": val_np,
        "svec": svec,
        "per_core": per_core,
    }


def _host_inputs(pp, x, l1_w, l1_b, bn_gamma, bn_beta, l2_w, l2_b, fp_w, fp_b):
    """Build the per-core in_maps (numpy only reshapes/pads, no math)."""
    x = np.ascontiguousarray(np.asarray(x, dtype=np.float32))
    iota = np.broadcast_to(np.arange(128, dtype=np.float32), (128, 128)).copy()
    ident = np.eye(128, dtype=np.float32)
    svec_pad = np.zeros((NCORES, 1, 3 * RPAD), dtype=np.float32)
    x_own = np.zeros((NCORES, RPAD, F), dtype=np.float32)
    for c in range(NCORES):
        sl = pp["svec"][:, c * RPC : (c + 1) * RPC]  # [3, RPC]
        for j in range(3):
            svec_pad[c, 0, j * RPAD : j * RPAD + RPC] = sl[j]
        x_own[c, :RPC] = x[c * RPC : (c + 1) * RPC]
    w1 = np.asarray(l1_w, np.float32)            # [3,128,128]
    b1 = np.asarray(l1_b, np.float32).reshape(1, 384)
    w2 = np.asarray(l2_w, np.float32)            # [3,384,128]
    b2T = np.ascontiguousarray(np.asarray(l2_b, np.float32).T)  # [128,3]
    fpw = np.asarray(fp_w, np.float32)           # [384,128]
    fpb = np.asarray(fp_b, np.float32).reshape(1, 128)
    gamc = np.ascontiguousarray(np.asarray(bn_gamma, np.float32).reshape(3, 128).T)  # [128,3]
    betc = np.ascontiguousarray(np.asarray(bn_beta, np.float32).reshape(3, 128).T)
    in_maps = []
    for c in range(NCORES):
        in_maps.append({
            "x_full": x,
            "x_own": x_own[c],
            "idx_all": pp["idx"][c],
            "rl_all": pp["rl"][c],
            "val_all": pp["val"][c],
            "iota": iota,
            "ident": ident,
            "w1": w1,
            "b1": b1,
            "w2": w2,
            "b2T": b2T,
            "fpw": fpw,
            "fpb": fpb,
            "gamc": gamc,
            "betc": betc,
            "svec": svec_pad[c],
        })
    return in_maps


# ----------------------------------------------------------------------------
# Numpy emulation of the device program (for fast validation)
# ----------------------------------------------------------------------------

def _emulate(pp, in_maps):
    outs = []
    x_full = in_maps[0]["x_full"]
    w1 = in_maps[0]["w1"]
    b1 = in_maps[0]["b1"].reshape(3, 128)
    w2 = in_maps[0]["w2"]
    b2T = in_maps[0]["b2T"]
    fpw = in_maps[0]["fpw"]
    fpb = in_maps[0]["fpb"].reshape(128)
    gamc = in_maps[0]["gamc"]
    betc = in_maps[0]["betc"]
    M = [w2[j] @ fpw[j * 128 : (j + 1) * 128] for j in range(3)]
    cj = [fpw[j * 128 : (j + 1) * 128].T @ b2T[:, j] for j in range(3)]

    def spmm(c, src_full):
        """returns [128 feats, RPAD] accumulated transposed tile outputs"""
        out = np.zeros((128, RPAD), dtype=np.float32)
        im = in_maps[c]
        for (t, h, co, nck) in pp["calls"]:
            L = nck * 128
            icols = im["idx_all"][:16, co * 8 : co * 8 + L // 16]
            idx = icols.T.reshape(-1).astype(np.int64)  # undo wrap
            rl = im["rl_all"][:, co : co + nck].T.reshape(-1)
            vv = im["val_all"][:, co : co + nck].T.reshape(-1)
            src = src_full[h * HALF :]
            G = src[idx]  # [L, 128]
            S = (rl[:, None] == np.arange(128)[None, :]) * vv[:, None]  # [L,128]
            out[:, t * TI : (t + 1) * TI] += G.T.astype(np.float32) @ S.astype(np.float32)
        return out

    # phase 1: y1 = A x  (per core slice, transposed)
    y1T = [spmm(c, x_full) for c in range(NCORES)]
    y1_full = np.concatenate([y1T[c].T[:RPC] for c in range(NCORES)])
    y2T = [spmm(c, y1_full) for c in range(NCORES)]
    hT = []
    for c in range(NCORES):
        im = in_maps[c]
        sv = im["svec"].reshape(3, RPAD)
        xT = im["x_own"].T  # [128, RPAD]
        h0 = w1[0].T @ xT + np.outer(b1[0], sv[0])
        h1 = w1[1].T @ y1T[c] + np.outer(b1[1], sv[1])
        h2 = w1[2].T @ y2T[c] + np.outer(b1[2], sv[2])
        hT.append(np.stack([h0, h1, h2]))  # [3,128,RPAD]
    # BN stats
    s = np.zeros((3, 128)); s2 = np.zeros((3, 128))
    for c in range(NCORES):
        s += hT[c][:, :, :RPC].sum(axis=2)
        s2 += (hT[c][:, :, :RPC] ** 2).sum(axis=2)
    mean = s * INV_N
    var = s2 * INV_N - mean ** 2
    scale = (var + BN_EPS) ** -0.5 * gamc.T
    shift = betc.T - mean * scale
    for c in range(NCORES):
        hT[c] = np.maximum(hT[c] * scale[:, :, None] + shift[:, :, None], 0.0)
    # phase 5: w = h M2
    w_full = np.zeros((N, 128), dtype=np.float32)
    for c in range(NCORES):
        wT = sum(M[2][kb * 128 : (kb + 1) * 128].T @ hT[c][kb] for kb in range(3))
        w_full[c * RPC : (c + 1) * RPC] = wT.T[:RPC]
    # phase 6: inner = h M1 + A w
    v1T = [spmm(c, w_full) for c in range(NCORES)]
    in_full = np.zeros((N, 128), dtype=np.float32)
    for c in range(NCORES):
        iT = v1T[c] + sum(M[1][kb * 128 : (kb + 1) * 128].T @ hT[c][kb] for kb in range(3))
        in_full[c * RPC : (c + 1) * RPC] = iT.T[:RPC]
    # phase 7/8: out = h M0 + A inner + bias
    v2T = [spmm(c, in_full) for c in range(NCORES)]
    out = np.zeros((N, 128), dtype=np.float32)
    for c in range(NCORES):
        sv = in_maps[c]["svec"].reshape(3, RPAD)
        oT = v2T[c] + sum(M[0][kb * 128 : (kb + 1) * 128].T @ hT[c][kb] for kb in range(3))
        oT += sum(np.outer(cj[j], sv[j]) for j in range(3)) + np.outer(fpb, sv[0])
        out[c * RPC : (c + 1) * RPC] = oT.T[:RPC]
    return out


# ----------------------------------------------------------------------------
# Bass device program
# ----------------------------------------------------------------------------

def _build_program(pp, use_bias):
    import concourse.bass as bass
    import concourse.bacc as bacc
    import concourse.tile as tile
    import concourse.mybir as mybir

    dt = mybir.dt
    Alu = mybir.AluOpType
    Act = mybir.ActivationFunctionType
    AX = mybir.AxisListType.X

    calls = pp["calls"]
    tile_chunks = pp["tile_chunks"]
    C_total = pp["C_total"]
    S_total = pp["S_total"]

    nc = bacc.Bacc("TRN2", target_bir_lowering=False, debug=False,
                   num_devices=NCORES)

    f32 = dt.float32
    din = lambda name, shape, d=f32: nc.dram_tensor(name, shape, d, kind="ExternalInput")
    x_full = din("x_full", [N, F])
    x_own = din("x_own", [RPAD, F])
    idx_all = din("idx_all", [128, S_total], dt.int16)
    rl_all = din("rl_all", [128, C_total])
    val_all = din("val_all", [128, C_total])
    iota_d = din("iota", [128, 128])
    ident_d = din("ident", [128, 128])
    w1_d = din("w1", [3, 128, 128])
    b1_d = din("b1", [1, 384])
    w2_d = din("w2", [3, 384, 128])
    b2T_d = din("b2T", [128, 3])
    fpw_d = din("fpw", [384, 128])
    fpb_d = din("fpb", [1, 128])
    gamc_d = din("gamc", [128, 3])
    betc_d = din("betc", [128, 3])
    svec_d = din("svec", [1, 3 * RPAD])
    outT_d = nc.dram_tensor("outT", [128, RPAD], f32, kind="ExternalOutput")

    RG = [list(range(NCORES))]

    with tile.TileContext(nc) as tc:
        with (
            tc.tile_pool(name="const", bufs=1) as cp,
            tc.tile_pool(name="hT", bufs=1) as hp,
            tc.tile_pool(name="gather", bufs=3) as gp,
            tc.tile_pool(name="sbuild", bufs=6) as sp,
            tc.tile_pool(name="work", bufs=6) as wp,
            tc.tile_pool(name="acc", bufs=2, space="PSUM") as accp,
            tc.tile_pool(name="trp", bufs=2, space="PSUM") as trp,
            tc.tile_pool(name="pp", bufs=2, space="PSUM") as ppp,
            tc.tile_pool(name="dram", bufs=1, space="DRAM") as dp,
        ):
            # ---- constants to SBUF ----
            idx_sb = cp.tile([128, S_total], dt.int16)
            nc.sync.dma_start(idx_sb[:], idx_all[:, :])
            rl_sb = cp.tile([128, C_total], f32)
            nc.sync.dma_start(rl_sb[:], rl_all[:, :])
            val_sb = cp.tile([128, C_total], f32)
            nc.sync.dma_start(val_sb[:], val_all[:, :])
            iota_sb = cp.tile([128, 128], f32)
            nc.sync.dma_start(iota_sb[:], iota_d[:, :])
            ident_sb = cp.tile([128, 128], f32)
            nc.sync.dma_start(ident_sb[:], ident_d[:, :])
            w1_sb = cp.tile([128, 384], f32)
            for j in range(3):
                nc.sync.dma_start(w1_sb[:, j * 128 : (j + 1) * 128], w1_d[j, :, :])
            b1_sb = cp.tile([1, 384], f32)
            nc.sync.dma_start(b1_sb[:], b1_d[:, :])
            w2_sb = cp.tile([128, 9 * 128], f32)
            for j in range(3):
                for kb in range(3):
                    nc.sync.dma_start(
                        w2_sb[:, (j * 3 + kb) * 128 : (j * 3 + kb + 1) * 128],
                        w2_d[j, kb * 128 : (kb + 1) * 128, :])
            b2T_sb = cp.tile([128, 3], f32)
            nc.sync.dma_start(b2T_sb[:], b2T_d[:, :])
            fpw_sb = cp.tile([128, 384], f32)
            for j in range(3):
                nc.sync.dma_start(fpw_sb[:, j * 128 : (j + 1) * 128],
                                  fpw_d[j * 128 : (j + 1) * 128, :])
            gam_sb = cp.tile([128, 3], f32)
            nc.sync.dma_start(gam_sb[:], gamc_d[:, :])
            bet_sb = cp.tile([128, 3], f32)
            nc.sync.dma_start(bet_sb[:], betc_d[:, :])
            if use_bias:
                svec_sb = cp.tile([1, 3 * RPAD], f32)
                nc.sync.dma_start(svec_sb[:], svec_d[:, :])
                cf_sb = cp.tile([1, 4 * 128], f32)   # c_j rows then fpb
                nc.sync.dma_start(cf_sb[:, 3 * 128 :], fpb_d[:, :])
            M_sb = cp.tile([128, 9 * 128], f32)  # M_j blocks, fin on partitions

            hT = [hp.tile([128, RPAD], f32, tag=f"hT{b}", name=f"hT{b}")
                  for b in range(3)]

            # ---- M_j = w2_j @ fp_w_j ; c_j = fp_w_j^T b2_j ----
            for j in range(3):
                for kb in range(3):
                    o = (j * 3 + kb) * 128
                    t_ps = trp.tile([128, 128], f32, tag="tr", name="t_ps")
                    nc.tensor.transpose(t_ps[:], w2_sb[:, o : o + 128], ident_sb[:])
                    w2T = wp.tile([128, 128], f32)
                    nc.scalar.copy(w2T[:], t_ps[:])
                    m_ps = ppp.tile([128, 128], f32, tag="pp", name="m_ps")
                    nc.tensor.matmul(m_ps[:], w2T[:], fpw_sb[:, j * 128 : (j + 1) * 128],
                                     start=True, stop=True)
                    nc.scalar.copy(M_sb[:, o : o + 128], m_ps[:])
                if use_bias:
                    c_ps = ppp.tile([1, 128], f32)
                    nc.tensor.matmul(c_ps[:], b2T_sb[:, j : j + 1],
                                     fpw_sb[:, j * 128 : (j + 1) * 128],
                                     start=True, stop=True)
                    nc.scalar.copy(cf_sb[:, j * 128 : (j + 1) * 128], c_ps[:])

            # ---- DRAM intermediates ----
            y1_own = dp.tile([RPC, F], f32)
            y1_full = dp.tile([N, F], f32)
            w_own = dp.tile([RPC, F], f32)
            w_full = dp.tile([N, F], f32)
            in_own = dp.tile([RPC, F], f32)
            in_full = dp.tile([N, F], f32)
            st_in = dp.tile([128, 6], f32)
            st_out = dp.tile([128, 6], f32)

            def emit_spmm_tile(t, src_lo_ap, src_hi_ap, extra_mms, my_calls):
                """Emit gathers + S-builds + accumulation matmuls for one row
                tile; extra_mms = [(lhsT_ap, rhs_ap), ...] appended to the
                same PSUM accumulation. Returns the PSUM tile."""
                ps = accp.tile([128, 128], f32, tag="acc")
                mms = []
                for (tt, h, co, nck) in my_calls:
                    L = nck * 128
                    G = gp.tile([128, MAXG * 128], f32, tag="G")
                    g3 = G[:, :L].rearrange("p (c f) -> p c f", f=128)
                    src = src_lo_ap if h == 0 else src_hi_ap
                    nc.gpsimd.dma_gather(g3, src, idx_sb[:, co * 8 : co * 8 + L // 16],
                                         L, L, 128, elem_step=128)
                    for k in range(nck):
                        S = sp.tile([128, 128], f32, tag="S")
                        nc.vector.tensor_scalar(
                            S[:], iota_sb[:], rl_sb[:, co + k : co + k + 1],
                            val_sb[:, co + k : co + k + 1], Alu.is_equal, Alu.mult)
                        mms.append((G[:, k * 128 : (k + 1) * 128], S[:]))
                mms.extend(extra_mms)
                nmm = len(mms)
                for i, (lh, rh) in enumerate(mms):
                    nc.tensor.matmul(ps[:], lh, rh, start=(i == 0), stop=(i == nmm - 1))
                return ps

            def tile_calls(t):
                return [cl for cl in calls if cl[0] == t]

            def rows_of(t):
                return min(RPC - t * TI, TI)

            def store_rows_major(t, src_sb, dram_tile):
                """src_sb [feat, row] tile -> rows-major dram slice"""
                rc = rows_of(t)
                tr = trp.tile([128, 128], f32, tag="tr")
                nc.tensor.transpose(tr[:], src_sb[:], ident_sb[:])
                r_sb = wp.tile([128, 128], f32, tag="rm")
                nc.scalar.copy(r_sb[:], tr[:])
                nc.sync.dma_start(dram_tile[t * TI : t * TI + rc, :], r_sb[:rc, :])

            def dense_into_hT(b, t, rhs_sb):
                """hT[b][:, tile t] = w1_b^T rhs + b1_b (x) svec_b"""
                ps = ppp.tile([128, 128], f32, tag="pp")
                if use_bias:
                    nc.tensor.matmul(ps[:], w1_sb[:, b * 128 : (b + 1) * 128],
                                     rhs_sb[:], start=True, stop=False)
                    nc.tensor.matmul(
                        ps[:], b1_sb[:, b * 128 : (b + 1) * 128],
                        svec_sb[:, b * RPAD + t * TI : b * RPAD + (t + 1) * TI],
                        start=False, stop=True)
                else:
                    nc.tensor.matmul(ps[:], w1_sb[:, b * 128 : (b + 1) * 128],
                                     rhs_sb[:], start=True, stop=True)
                nc.scalar.copy(hT[b][:, t * TI : (t + 1) * TI], ps[:])

            # ---- phase 1b: p0 = x W0 (+bias) ----
            for t in range(NT):
                xt = wp.tile([128, 128], f32, tag="xt")
                nc.sync.dma_start(xt[:], x_own[t * TI : (t + 1) * TI, :])
                tr = trp.tile([128, 128], f32, tag="tr")
                nc.tensor.transpose(tr[:], xt[:], ident_sb[:])
                xT = wp.tile([128, 128], f32, tag="xT")
                nc.scalar.copy(xT[:], tr[:])
                dense_into_hT(0, t, xT)

            # ---- phase 1: y1 = A x ; p1 ; stage y1 for AG ----
            for t in range(NT):
                ps = emit_spmm_tile(t, x_full[:, :], x_full[HALF:, :], [], tile_calls(t))
                y1t = wp.tile([128, 128], f32, tag="y1t")
                nc.scalar.copy(y1t[:], ps[:])
                dense_into_hT(1, t, y1t)
                store_rows_major(t, y1t, y1_own)
            nc.gpsimd.collective_compute(
                "AllGather", Alu.bypass, replica_groups=RG,
                ins=[y1_own.opt()], outs=[y1_full.opt()])

            # ---- phase 2: y2 = A y1 ; p2 ----
            for t in range(NT):
                ps = emit_spmm_tile(t, y1_full[:, :], y1_full[HALF:, :], [], tile_calls(t))
                y2t = wp.tile([128, 128], f32, tag="y2t")
                nc.scalar.copy(y2t[:], ps[:])
                dense_into_hT(2, t, y2t)

            # ---- phase 3: BN + ReLU ----
            st_sb = cp.tile([128, 6], f32)
            st13 = cp.tile([128, 13], f32)
            NB = (RPC + 511) // 512  # 13 blocks of <=512 rows
            for b in range(3):
                nc.vector.tensor_reduce(st_sb[:, b : b + 1], hT[b][:, :RPC], AX, Alu.add)
                for k in range(NB):
                    c0 = k * 512
                    c1 = min(RPC, c0 + 512)
                    sq = wp.tile([128, 512], f32, tag="sq")
                    nc.scalar.activation(sq[:, : c1 - c0], hT[b][:, c0:c1], Act.Square)
                    nc.vector.tensor_reduce(st13[:, k : k + 1], sq[:, : c1 - c0],
                                            AX, Alu.add)
                nc.vector.tensor_reduce(st_sb[:, 3 + b : 4 + b], st13[:, :NB],
                                        AX, Alu.add)
            nc.sync.dma_start(st_in[:, :], st_sb[:])
            nc.gpsimd.collective_compute(
                "AllReduce", Alu.add, replica_groups=RG,
                ins=[st_in.opt()], outs=[st_out.opt()])
            st2_sb = cp.tile([128, 6], f32)
            nc.sync.dma_start(st2_sb[:], st_out[:, :])
            bnw = cp.tile([128, 18], f32)  # mean, ex2, var, rstd, scale, shift x3
            for b in range(3):
                mn = bnw[:, b : b + 1]
                ex2 = bnw[:, 3 + b : 4 + b]
                var = bnw[:, 6 + b : 7 + b]
                rs = bnw[:, 9 + b : 10 + b]
                sc = bnw[:, 12 + b : 13 + b]
                sh = bnw[:, 15 + b : 16 + b]
                nc.vector.tensor_scalar(mn, st2_sb[:, b : b + 1], INV_N, None, Alu.mult)
                nc.vector.tensor_scalar(ex2, st2_sb[:, 3 + b : 4 + b], INV_N, None, Alu.mult)
                nc.vector.tensor_tensor(var, mn, mn, Alu.mult)
                nc.vector.tensor_tensor(var, ex2, var, Alu.subtract)
                nc.vector.tensor_scalar(var, var, BN_EPS, None, Alu.add)
                nc.scalar.activation(rs, var, Act.Sqrt)
                nc.vector.reciprocal(rs, rs)
                nc.vector.tensor_tensor(sc, rs, gam_sb[:, b : b + 1], Alu.mult)
                nc.vector.tensor_tensor(sh, mn, sc, Alu.mult)
                nc.vector.tensor_tensor(sh, bet_sb[:, b : b + 1], sh, Alu.subtract)
                nc.vector.tensor_scalar(hT[b][:], hT[b][:], sc, sh, Alu.mult, Alu.add)
                nc.scalar.activation(hT[b][:], hT[b][:], Act.Relu)

            # ---- phase 5: w = h M2 -> AG ----
            def hmj_mms(j, t):
                return [(M_sb[:, (j * 3 + kb) * 128 : (j * 3 + kb + 1) * 128],
                         hT[kb][:, t * TI : (t + 1) * TI]) for kb in range(3)]

            for t in range(NT):
                ps = ppp.tile([128, 128], f32, tag="pp")
                for i, (lh, rh) in enumerate(hmj_mms(2, t)):
                    nc.tensor.matmul(ps[:], lh, rh, start=(i == 0), stop=(i == 2))
                wt = wp.tile([128, 128], f32, tag="wt")
                nc.scalar.copy(wt[:], ps[:])
                store_rows_major(t, wt, w_own)
            nc.gpsimd.collective_compute(
                "AllGather", Alu.bypass, replica_groups=RG,
                ins=[w_own.opt()], outs=[w_full.opt()])

            # ---- phase 6: inner = h M1 + A w -> AG ----
            for t in range(NT):
                ps = emit_spmm_tile(t, w_full[:, :], w_full[HALF:, :],
                                    hmj_mms(1, t), tile_calls(t))
                it = wp.tile([128, 128], f32, tag="it")
                nc.scalar.copy(it[:], ps[:])
                store_rows_major(t, it, in_own)
            nc.gpsimd.collective_compute(
                "AllGather", Alu.bypass, replica_groups=RG,
                ins=[in_own.opt()], outs=[in_full.opt()])

            # ---- phase 7/8: out = h M0 + A inner + biases ----
            for t in range(NT):
                extra = hmj_mms(0, t)
                if use_bias:
                    for j in range(3):
                        extra.append(
                            (cf_sb[:, j * 128 : (j + 1) * 128],
                             svec_sb[:, j * RPAD + t * TI : j * RPAD + (t + 1) * TI]))
                    extra.append((cf_sb[:, 3 * 128 :],
                                  svec_sb[:, t * TI : (t + 1) * TI]))
                ps = emit_spmm_tile(t, in_full[:, :], in_full[HALF:, :],
                                    extra, tile_calls(t))
                ot = wp.tile([128, 128], f32, tag="ot")
                nc.scalar.copy(ot[:], ps[:])
                nc.sync.dma_start(outT_d[:, t * TI : (t + 1) * TI], ot[:])

    nc.compile()
    return nc


# ----------------------------------------------------------------------------
# Entry point
# ----------------------------------------------------------------------------

def _install_ntff_hook_stub():
    """The agent image's antenv lacks axon_hooks; provide it so
    run_bass_kernel_spmd(trace=True) can NTFF-profile via libaxon_pjrt."""
    import sys
    import types
    if "antenv.axon_hooks" in sys.modules:
        return
    try:
        from trn_agent_boot.trn_boot import _ntff_profile_via_ctypes
        hook = _ntff_profile_via_ctypes("/opt/axon/libaxon_pjrt.so")
    except Exception:
        return
    mod = types.ModuleType("antenv.axon_hooks")
    mod._hook = hook
    mod.get_axon_ntff_profile_hook = lambda: mod._hook
    mod.set_axon_ntff_profile_hook = lambda h: setattr(mod, "_hook", h)
    sys.modules["antenv.axon_hooks"] = mod
    try:
        import antenv
        antenv.axon_hooks = mod
    except Exception:
        pass


def kernel(x, edge_index, l1_w, l1_b, bn_gamma, bn_beta, l2_w, l2_b, fp_w, fp_b):
    global LAST_EXEC_NS
    pp = _preprocess(edge_index)
    in_maps = _host_inputs(pp, x, l1_w, l1_b, bn_gamma, bn_beta,
                           l2_w, l2_b, fp_w, fp_b)
    if os.environ.get("KERNEL_EMULATE"):
        return _emulate(pp, in_maps)

    use_bias = bool(np.any(np.asarray(l1_b)) or np.any(np.asarray(l2_b))
                    or np.any(np.asarray(fp_b)))
    from concourse import bass_utils
    key = "prog"
    if key not in _COMPILED:
        _COMPILED[key] = _build_program(pp, use_bias)
    nc = _COMPILED[key]
    trace = bool(os.environ.get("KERNEL_TRACE"))
    if trace:
        _install_ntff_hook_stub()
    try:
        res = bass_utils.run_bass_kernel_spmd(
            nc, in_maps, core_ids=list(range(NCORES)), trace=trace)
    except Exception:
        if not trace:
            raise
        res = bass_utils.run_bass_kernel_spmd(
            nc, in_maps, core_ids=list(range(NCORES)), trace=False)
    LAST_EXEC_NS = res.exec_time_ns
    out = np.concatenate(
        [np.asarray(res.results[c]["outT"]).T[:RPC] for c in range(NCORES)])
    return out.astype(np.float32)
